# revision 1
# baseline (speedup 1.0000x reference)
"""Trainium2 kernel: y = relu((x - pb) @ W + b) with per-row top-K threshold masking.

Strategy (per spec sharding hint): data-parallel over rows across 8 cores.
Each core computes its row shard with a 3-pass bf16 matmul decomposition
(x_hi@W_hi + x_hi@W_lo + x_lo@W_hi, f32 PSUM accumulation, ~1e-5 accurate),
then finds each row's K-th largest activation by a fused count binary search
on DVE (tensor_scalar is_ge + accumulate), which converges to 1 ulp and
reproduces jax.lax.top_k threshold masking exactly (ties included).
"""
import sys
sys.path.insert(0, "/opt/trn_rl_repo")

import numpy as np
import concourse.bass as bass
import concourse.bacc as bacc
import concourse.mybir as mybir
from concourse.tile import TileContext
from concourse.masks import make_identity

F32 = mybir.dt.float32
BF16 = mybir.dt.bfloat16
FP8 = mybir.dt.float8e4

# full problem dims (hardcoded; kernel.py must be self-contained)
B_FULL, D_IN, N_FEAT, K_TOP = 16384, 4096, 4096, 128
N_CORES = 8


def build_nc(B_core, D, F, K, n_iters=20, super_size=2, fb=512, debug_acts=False, repeat=1):
    assert B_core % 128 == 0 and D % 128 == 0 and F % fb == 0
    nc = bacc.Bacc("TRN2", target_bir_lowering=False, debug=True)
    x = nc.dram_tensor("x", [B_core, D], F32, kind="ExternalInput")
    w = nc.dram_tensor("w", [D, F], F32, kind="ExternalInput")
    out = nc.dram_tensor("out", [B_core, F], F32, kind="ExternalOutput")
    acts_dbg = None
    if debug_acts:
        acts_dbg = nc.dram_tensor("acts_dbg", [B_core, F], F32, kind="ExternalOutput")

    n_r = B_core // 128   # row blocks
    n_d = D // 128        # contraction blocks
    n_fb = F // fb        # feature blocks
    supers = [list(range(i, min(i + super_size, n_r)))
              for i in range(0, n_r, super_size)]

    with TileContext(nc) as tc:
        from contextlib import ExitStack
        ctx = ExitStack()
        cpool = ctx.enter_context(tc.tile_pool(name="const", bufs=1))
        dpool = ctx.enter_context(tc.tile_pool(name="wdram", bufs=1, space="DRAM"))
        xr_pool = ctx.enter_context(tc.tile_pool(name="xr", bufs=1))
        xsp_pool = ctx.enter_context(tc.tile_pool(name="xsp", bufs=2 * super_size))
        xt_pool = ctx.enter_context(tc.tile_pool(name="xt", bufs=4 * n_d))
        w_pool = ctx.enter_context(tc.tile_pool(name="wp", bufs=4))
        wsrc_pool = ctx.enter_context(tc.tile_pool(name="wsrc", bufs=2))
        acts_pool = ctx.enter_context(tc.tile_pool(name="acts", bufs=super_size + 1))
        scr_pool = ctx.enter_context(tc.tile_pool(name="scr", bufs=1))
        scra_pool = ctx.enter_context(tc.tile_pool(name="scra", bufs=1))
        sm_pool = ctx.enter_context(tc.tile_pool(name="sm", bufs=4 * 6))
        mm_pool = ctx.enter_context(tc.tile_pool(name="mm", bufs=super_size + 2, space="PSUM"))
        tp_pool = ctx.enter_context(tc.tile_pool(name="tp", bufs=4, space="PSUM"))

        ident = cpool.tile([128, 128], BF16)
        make_identity(nc, ident[:])

        wh_d = dpool.tile([D, F], BF16)
        wl_d = dpool.tile([D, F], BF16)

        sched = [(si2 == 0 and rep == 0, sup2) for rep in range(repeat)
                 for si2, sup2 in enumerate(supers)]
        for do_split, sup in sched:
            ns = len(sup)
            # ---- split x rows into bf16 hi/lo, then transpose via PE ----
            xh_rows, xl_rows = [], []
            for r in sup:
                xr = xr_pool.tile([128, D], F32)
                nc.sync.dma_start(out=xr[:], in_=x[r * 128:(r + 1) * 128, :])
                xh = xsp_pool.tile([128, D], BF16, tag="xsp")
                nc.vector.tensor_copy(xh[:], xr[:])
                xl = xsp_pool.tile([128, D], BF16, tag="xsp")
                nc.vector.tensor_tensor(out=xl[:], in0=xr[:], in1=xh[:],
                                        op=mybir.AluOpType.subtract)
                xh_rows.append(xh)
                xl_rows.append(xl)
            xhT, xlT = [], []
            for db in range(n_d):
                dsl = slice(db * 128, (db + 1) * 128)
                ph = tp_pool.tile([128, ns * 128], BF16, tag="tp")
                for i in range(ns):
                    nc.tensor.transpose(ph[:, i * 128:(i + 1) * 128],
                                        xh_rows[i][:, dsl], ident[:])
                th = xt_pool.tile([128, ns * 128], BF16, tag="xt")
                nc.scalar.copy(th[:], ph[:])
                xhT.append(th)
                pl = tp_pool.tile([128, ns * 128], BF16, tag="tp")
                for i in range(ns):
                    nc.tensor.transpose(pl[:, i * 128:(i + 1) * 128],
                                        xl_rows[i][:, dsl], ident[:])
                tl = xt_pool.tile([128, ns * 128], BF16, tag="xt")
                nc.scalar.copy(tl[:], pl[:])
                xlT.append(tl)

            # ---- 3-pass matmul over feature blocks ----
            acts = [acts_pool.tile([128, F], F32, tag="acts", name=f"acts{_i}") for _i in range(ns)]
            CH = 4  # d-blocks per W chunk DMA
            for f in range(n_fb):
                fsl = slice(f * fb, (f + 1) * fb)
                pms = [mm_pool.tile([128, fb], F32, tag="mm", name=f"pm{_i}") for _i in range(ns)]
                for dbc in range(n_d // CH):
                    d0 = dbc * CH * 128
                    dcsl = slice(d0, d0 + CH * 128)
                    # DRAM view [128 part, CH, fb]
                    wv = w[dcsl, fsl].rearrange("(c p) f -> p c f", p=128)
                    whv = wh_d[dcsl, fsl].rearrange("(c p) f -> p c f", p=128)
                    wlv = wl_d[dcsl, fsl].rearrange("(c p) f -> p c f", p=128)
                    if do_split:
                        # split W on the fly (half-chunks); cache bf16 pieces in DRAM
                        wh_sb = w_pool.tile([128, CH, fb], BF16, tag="wp")
                        wl_sb = w_pool.tile([128, CH, fb], BF16, tag="wp")
                        H = CH // 2
                        for hh in range(2):
                            hsl = slice(hh * H, (hh + 1) * H)
                            wsrc = wsrc_pool.tile([128, H, fb], F32)
                            nc.sync.dma_start(out=wsrc[:], in_=wv[:, hsl, :])
                            nc.vector.tensor_copy(wh_sb[:, hsl, :], wsrc[:])
                            nc.vector.tensor_tensor(out=wl_sb[:, hsl, :], in0=wsrc[:],
                                                    in1=wh_sb[:, hsl, :],
                                                    op=mybir.AluOpType.subtract)
                        nc.sync.dma_start(out=whv, in_=wh_sb[:])
                        nc.sync.dma_start(out=wlv, in_=wl_sb[:])
                    else:
                        wh_sb = w_pool.tile([128, CH, fb], BF16, tag="wp")
                        nc.sync.dma_start(out=wh_sb[:], in_=whv)
                        wl_sb = w_pool.tile([128, CH, fb], BF16, tag="wp")
                        nc.sync.dma_start(out=wl_sb[:], in_=wlv)
                    for j in range(CH):
                        db = dbc * CH + j
                        whj = wh_sb[:, j, :]
                        wlj = wl_sb[:, j, :]
                        last = db == n_d - 1
                        for i in range(ns):
                            isl = slice(i * 128, (i + 1) * 128)
                            nc.tensor.matmul(pms[i][:], xhT[db][:, isl], whj,
                                             start=(db == 0), stop=False)
                            nc.tensor.matmul(pms[i][:], xhT[db][:, isl], wlj,
                                             start=False, stop=False)
                            nc.tensor.matmul(pms[i][:], xlT[db][:, isl], whj,
                                             start=False, stop=last)
                for i in range(ns):
                    nc.scalar.activation(acts[i][:, fsl], pms[i][:],
                                         mybir.ActivationFunctionType.Relu)

            if debug_acts:
                for i, r in enumerate(sup):
                    nc.sync.dma_start(out=acts_dbg[r * 128:(r + 1) * 128, :],
                                      in_=acts[i][:])

            # ---- per-row K-th largest via count binary search ----
            # state: lo (threshold lower bound), wdt (interval width); hi = lo + wdt
            # invariant: count(acts >= lo) >= K, count(acts >= lo + wdt) < K
            lo = sm_pool.tile([128, ns], F32, tag="sm")
            nc.vector.memset(lo[:], 0.0)
            wdt = sm_pool.tile([128, ns], F32, tag="sm")
            for i in range(ns):
                nc.vector.reduce_max(out=wdt[:, i:i + 1], in_=acts[i][:],
                                     axis=mybir.AxisListType.X)
            nc.vector.tensor_scalar(wdt[:], wdt[:], 1.0001, 1e-20,
                        op0=mybir.AluOpType.mult, op1=mybir.AluOpType.add)
            mid = sm_pool.tile([128, ns], F32, tag="sm")
            nc.vector.tensor_scalar_mul(mid[:], wdt[:], 0.5)
            cnt = sm_pool.tile([128, ns], F32, tag="sm")
            tgw = sm_pool.tile([128, ns], F32, tag="sm")
            for it in range(n_iters):
                for i in range(ns):
                    if i % 2 == 0:
                        # DVE: exact count of acts >= mid
                        scr = scr_pool.tile([128, F], FP8, tag="scr")
                        nc.vector.tensor_scalar(scr[:], acts[i][:], mid[:, i:i + 1],
                                                None, op0=mybir.AluOpType.is_ge,
                                                op1=mybir.AluOpType.add,
                                                accum_out=cnt[:, i:i + 1])
                    else:
                        # ACT: S' = sum(sign(mid - a)) = B - A (ties -> 0);
                        # count_eff = (F - S')/2 = A + Z/2 -- exact for the
                        # (cnt >= K - 0.75) test except two exact mid-hits.
                        scr2 = scra_pool.tile([128, F], FP8, tag="scra")
                        nc.scalar.activation(scr2[:], acts[i][:],
                                             mybir.ActivationFunctionType.Sign,
                                             bias=mid[:, i:i + 1], scale=-1.0,
                                             accum_out=cnt[:, i:i + 1])
                        nc.vector.tensor_scalar(cnt[:, i:i + 1], cnt[:, i:i + 1],
                                                -0.5, float(F) / 2.0,
                                                op0=mybir.AluOpType.mult,
                                                op1=mybir.AluOpType.add)
                # wdt *= 0.5 ; lo += (cnt >= K - 0.75) * wdt ; mid = 0.5*wdt + lo
                nc.vector.tensor_scalar_mul(wdt[:], wdt[:], 0.5)
                nc.vector.scalar_tensor_tensor(out=tgw[:], in0=cnt[:],
                                               scalar=float(K) - 0.75,
                                               in1=wdt[:], op0=mybir.AluOpType.is_ge,
                                               op1=mybir.AluOpType.mult)
                nc.vector.tensor_tensor(out=lo[:], in0=lo[:], in1=tgw[:],
                                        op=mybir.AluOpType.add)
                if it != n_iters - 1:
                    nc.vector.scalar_tensor_tensor(out=mid[:], in0=wdt[:], scalar=0.5,
                                                   in1=lo[:], op0=mybir.AluOpType.mult,
                                                   op1=mybir.AluOpType.add)
            # ---- apply mask: out = acts * (acts >= lo) ----
            for i, r in enumerate(sup):
                nc.vector.scalar_tensor_tensor(out=acts[i][:], in0=acts[i][:],
                                               scalar=lo[:, i:i + 1], in1=acts[i][:],
                                               op0=mybir.AluOpType.is_ge,
                                               op1=mybir.AluOpType.mult)
                nc.sync.dma_start(out=out[r * 128:(r + 1) * 128, :], in_=acts[i][:])
        ctx.close()

    nc.finalize()
    return nc


_NC_CACHE = {}


def _get_nc(key):
    if key not in _NC_CACHE:
        _NC_CACHE[key] = build_nc(*key)
    return _NC_CACHE[key]


def kernel(x, preencoder_bias, W_enc, b_enc):
    from concourse.bass_utils import run_bass_kernel_spmd
    x = np.asarray(x, dtype=np.float32)
    W = np.asarray(W_enc, dtype=np.float32)
    pb = np.asarray(preencoder_bias, dtype=np.float32)
    b = np.asarray(b_enc, dtype=np.float32)

    B, D = x.shape
    F = W.shape[1]
    assert (B, D, F) == (B_FULL, D_IN, N_FEAT)
    # fold biases: (x - pb) @ W + b == x @ W + (b - pb @ W)
    c = (b - pb @ W).astype(np.float32)
    if np.any(c != 0.0):
        # exact: augment the contraction with one extra 128-block where
        # x_aug[:, D] = 1 and W_aug[D, :] = c (rest zeros)
        pad = 512  # keep D a multiple of the CH*128 W-chunking
        x_aug = np.zeros((B, D + pad), dtype=np.float32)
        x_aug[:, :D] = x
        x_aug[:, D] = 1.0
        W_aug = np.zeros((D + pad, F), dtype=np.float32)
        W_aug[:D] = W
        W_aug[D] = c
        x, W, D = x_aug, W_aug, D + pad

    B_core = B // N_CORES
    nc = _get_nc((B_core, D, F, K_TOP))
    in_maps = [{"x": np.ascontiguousarray(x[i * B_core:(i + 1) * B_core]), "w": W}
               for i in range(N_CORES)]
    res = run_bass_kernel_spmd(nc, in_maps, core_ids=list(range(N_CORES)))
    return np.concatenate([res.results[i]["out"] for i in range(N_CORES)], axis=0)



# revision 9
# speedup vs baseline: 3.7131x; 3.7131x over previous
"""Trainium2 kernel: y = relu((x - pb) @ W + b) with per-row top-K threshold masking.

Strategy: data-parallel over rows across 8 cores (per spec hint).

Matmul: SINGLE PASS in float32r — the PE reads 4-byte fp32 and truncates to
fp22 (e10m11) internally, running at bf16 speed (1 cycle/row for N>=256).
End-to-end rel err of the fp22 quantization on these inputs is 0.0167
(simulated exactly; the 2e-2 gate passes). This replaces the baseline's
3-pass bf16 decomposition: 3x less PE work, no split/convert overhead.

x is pre-transposed on the host (xt = x.T per core shard), so no PE
transposes or staging are needed; W streams from DRAM f32 once per
512-row group (4 streams total vs the baseline's 8).

Top-K threshold per row via count binary search (16 iters) on the f32 acts,
split across DVE (2 row-tiles), ACT via a Sign-accumulate trick (1 tile),
and GpSimd (1 tile), overlapped with the next group's matmuls.
"""
import sys
sys.path.insert(0, "/opt/trn_rl_repo")

import numpy as np
import concourse.bass as bass
import concourse.bacc as bacc
import concourse.mybir as mybir
from concourse.tile import TileContext

F32 = mybir.dt.float32
F32R = mybir.dt.float32r
FP8 = mybir.dt.float8e4

# full problem dims (hardcoded; kernel.py must be self-contained)
B_FULL, D_IN, N_FEAT, K_TOP = 16384, 4096, 4096, 128
N_CORES = 8


def build_nc(B_core, D, F, K, n_iters=16, rt=4, fb=512, repeat=1):
    assert B_core % (128 * rt) == 0 and D % 256 == 0 and F % fb == 0
    nc = bacc.Bacc("TRN2", target_bir_lowering=False, debug=True)
    xt = nc.dram_tensor("xt", [D, B_core], F32R, kind="ExternalInput")
    w = nc.dram_tensor("w", [D, F], F32R, kind="ExternalInput")
    out = nc.dram_tensor("out", [B_core, F], F32, kind="ExternalOutput")

    n_r = B_core // 128   # row tiles (16)
    n_d = D // 128        # contraction blocks (32)
    n_fb = F // fb        # feature blocks (8)
    n_g = n_r // rt       # row groups (4)
    CH = 2                # d-blocks per W DMA chunk (512 KB)

    with TileContext(nc) as tc:
        from contextlib import ExitStack
        ctx = ExitStack()
        xt_pool = ctx.enter_context(tc.tile_pool(name="xtp", bufs=n_d + 2))
        w_pool = ctx.enter_context(tc.tile_pool(name="wp", bufs=3))
        acts_pool = ctx.enter_context(tc.tile_pool(name="acts", bufs=rt + 3))
        scr_pool = ctx.enter_context(tc.tile_pool(name="scr", bufs=1))
        scra_pool = ctx.enter_context(tc.tile_pool(name="scra", bufs=1))
        scrg_pool = ctx.enter_context(tc.tile_pool(name="scrg", bufs=1))
        sm_pool = ctx.enter_context(tc.tile_pool(name="sm", bufs=2 * 6))
        mm_pool = ctx.enter_context(tc.tile_pool(name="mm", bufs=8, space="PSUM"))

        for rep in range(repeat):
            for g in range(n_g):
                r0 = g * rt
                rsl = slice(r0 * 128, (r0 + rt) * 128)
                # ---- stream this group's x^T tiles: [128 d, rt*128 rows] ----
                xts = []
                for db in range(n_d):
                    xtile = xt_pool.tile([128, rt * 128], F32R, tag="xt")
                    nc.sync.dma_start(out=xtile[:], in_=xt[db * 128:(db + 1) * 128, rsl])
                    xts.append(xtile)

                acts = [acts_pool.tile([128, F], F32, tag="acts", name=f"acts{_i}")
                        for _i in range(rt)]

                # ---- single-pass fp32r matmul over feature blocks ----
                for f in range(n_fb):
                    fsl = slice(f * fb, (f + 1) * fb)
                    pms = [mm_pool.tile([128, fb], F32, tag="mm", name=f"pm{_i}")
                           for _i in range(rt)]
                    for dc in range(n_d // CH):
                        d0 = dc * CH * 128
                        wv = w[d0:d0 + CH * 128, fsl].rearrange("(c p) f -> p c f", p=128)
                        wc = w_pool.tile([128, CH, fb], F32R, tag="wp")
                        nc.sync.dma_start(out=wc[:], in_=wv)
                        for j in range(CH):
                            db = dc * CH + j
                            for i in range(rt):
                                isl = slice(i * 128, (i + 1) * 128)
                                nc.tensor.matmul(pms[i][:],
                                                 xts[db][:, isl],
                                                 wc[:, j, :],
                                                 start=(db == 0), stop=(db == n_d - 1))
                    for i in range(rt):
                        nc.scalar.activation(acts[i][:, fsl], pms[i][:],
                                             mybir.ActivationFunctionType.Relu)

                # ---- per-row K-th largest via count binary search ----
                # invariant: count(acts >= lo) >= K, count(acts >= lo + wdt) < K
                lo = sm_pool.tile([128, rt], F32, tag="sm")
                nc.vector.memset(lo[:], 0.0)
                wdt = sm_pool.tile([128, rt], F32, tag="sm")
                for i in range(rt):
                    nc.vector.reduce_max(out=wdt[:, i:i + 1], in_=acts[i][:],
                                         axis=mybir.AxisListType.X)
                nc.vector.tensor_scalar(wdt[:], wdt[:], 1.0001, 1e-20,
                                        op0=mybir.AluOpType.mult,
                                        op1=mybir.AluOpType.add)
                mid = sm_pool.tile([128, rt], F32, tag="sm")
                nc.vector.tensor_scalar_mul(mid[:], wdt[:], 0.5)
                cnt = sm_pool.tile([128, rt + 1], F32, tag="sm")
                tgw = sm_pool.tile([128, rt], F32, tag="sm")
                SP = 1920  # tile-3 split point balancing DVE vs ACT rates
                for it in range(n_iters):
                    # tiles 0,1 + tile 3's first SP cols on DVE: count acts >= mid
                    for i in (0, 1):
                        scr = scr_pool.tile([128, F], FP8, tag="scr")
                        nc.vector.tensor_scalar(scr[:], acts[i][:], mid[:, i:i + 1],
                                                None, op0=mybir.AluOpType.is_ge,
                                                op1=mybir.AluOpType.add,
                                                accum_out=cnt[:, i:i + 1])
                    scr3 = scrg_pool.tile([128, SP], FP8, tag="scrg")
                    nc.vector.tensor_scalar(scr3[:], acts[3][:, :SP], mid[:, 3:4],
                                            None, op0=mybir.AluOpType.is_ge,
                                            op1=mybir.AluOpType.add,
                                            accum_out=cnt[:, 3:4])
                    # tile 2 (+ tile 3 tail) on ACT:
                    # S = sum(sign(mid - a)); count_eff = (n - S)/2
                    scr2 = scra_pool.tile([128, F], FP8, tag="scra")
                    nc.scalar.activation(scr2[:], acts[2][:],
                                         mybir.ActivationFunctionType.Sign,
                                         bias=mid[:, 2:3], scale=-1.0,
                                         accum_out=cnt[:, 2:3])
                    scr4 = scra_pool.tile([128, F - SP], FP8, tag="scra")
                    nc.scalar.activation(scr4[:], acts[3][:, SP:],
                                         mybir.ActivationFunctionType.Sign,
                                         bias=mid[:, 3:4], scale=-1.0,
                                         accum_out=cnt[:, rt:rt + 1])
                    nc.vector.tensor_scalar(cnt[:, 2:3], cnt[:, 2:3],
                                            -0.5, float(F) / 2.0,
                                            op0=mybir.AluOpType.mult,
                                            op1=mybir.AluOpType.add)
                    # fold tile-3 tail: cnt3 += (n_tail - S_tail)/2
                    nc.vector.tensor_scalar(cnt[:, rt:rt + 1], cnt[:, rt:rt + 1],
                                            -0.5, float(F - SP) / 2.0,
                                            op0=mybir.AluOpType.mult,
                                            op1=mybir.AluOpType.add)
                    nc.vector.tensor_tensor(out=cnt[:, 3:4], in0=cnt[:, 3:4],
                                            in1=cnt[:, rt:rt + 1],
                                            op=mybir.AluOpType.add)
                    # wdt *= 0.5 ; lo += (cnt >= K - 0.75) * wdt ; mid = 0.5*wdt + lo
                    nc.vector.tensor_scalar_mul(wdt[:], wdt[:], 0.5)
                    nc.vector.scalar_tensor_tensor(out=tgw[:], in0=cnt[:, :rt],
                                                   scalar=float(K) - 0.75,
                                                   in1=wdt[:],
                                                   op0=mybir.AluOpType.is_ge,
                                                   op1=mybir.AluOpType.mult)
                    nc.vector.tensor_tensor(out=lo[:], in0=lo[:], in1=tgw[:],
                                            op=mybir.AluOpType.add)
                    if it != n_iters - 1:
                        nc.vector.scalar_tensor_tensor(out=mid[:], in0=wdt[:],
                                                       scalar=0.5, in1=lo[:],
                                                       op0=mybir.AluOpType.mult,
                                                       op1=mybir.AluOpType.add)
                # ---- apply mask: out = acts * (acts >= lo), then write out ----
                for i in range(rt):
                    nc.vector.scalar_tensor_tensor(out=acts[i][:], in0=acts[i][:],
                                                   scalar=lo[:, i:i + 1],
                                                   in1=acts[i][:],
                                                   op0=mybir.AluOpType.is_ge,
                                                   op1=mybir.AluOpType.mult)
                    r = r0 + i
                    nc.sync.dma_start(out=out[r * 128:(r + 1) * 128, :],
                                      in_=acts[i][:])
        ctx.close()

    nc.finalize()
    return nc


_NC_CACHE = {}


def _get_nc(key):
    if key not in _NC_CACHE:
        _NC_CACHE[key] = build_nc(*key)
    return _NC_CACHE[key]


def _round_fp22(a):
    """Round f32 to nearest-even on the fp22 (e10m11) grid the PE uses, so the
    on-device float32r truncation is a no-op and quantization is RN not RTZ."""
    v = np.ascontiguousarray(a).view(np.uint32)
    r = ((v >> 12) & np.uint32(1)) + np.uint32(0x7FF)
    return ((v + r) & np.uint32(0xFFFFF000)).view(np.float32)


def kernel(x, preencoder_bias, W_enc, b_enc):
    from concourse.bass_utils import run_bass_kernel_spmd
    x = np.asarray(x, dtype=np.float32)
    W = np.asarray(W_enc, dtype=np.float32)
    pb = np.asarray(preencoder_bias, dtype=np.float32)
    b = np.asarray(b_enc, dtype=np.float32)

    B, D = x.shape
    F = W.shape[1]
    assert (B, D, F) == (B_FULL, D_IN, N_FEAT)
    # fold biases: (x - pb) @ W + b == x @ W + (b - pb @ W)
    c = (b - pb @ W).astype(np.float32)
    if np.any(c != 0.0):
        # exact: augment the contraction with one extra row block where
        # xT_aug[D, :] = 1 and W_aug[D, :] = c (rest zeros)
        pad = 256
        xT = np.zeros((D + pad, B), dtype=np.float32)
        xT[:D] = x.T
        xT[D] = 1.0
        W_aug = np.zeros((D + pad, F), dtype=np.float32)
        W_aug[:D] = W
        W_aug[D] = c
        W, D = W_aug, D + pad
    else:
        xT = np.ascontiguousarray(x.T)

    xT = _round_fp22(xT)
    W = _round_fp22(W)
    B_core = B // N_CORES
    nc = _get_nc((B_core, D, F, K_TOP))
    in_maps = [{"xt": np.ascontiguousarray(xT[:, i * B_core:(i + 1) * B_core]),
                "w": W}
               for i in range(N_CORES)]
    res = run_bass_kernel_spmd(nc, in_maps, core_ids=list(range(N_CORES)))
    return np.concatenate([res.results[i]["out"] for i in range(N_CORES)], axis=0)
